# revision 7
# baseline (speedup 1.0000x reference)
"""ChatGLM3 attention (B=2, S=2048, H=4096, 32 q-heads / 2 kv-heads, D=128)
on 8 Trainium2 NeuronCores.

Sharding: core c = 4*b + tp  (b in {0,1} data-parallel over batch,
tp in {0..3} tensor-parallel over heads). Each core computes the QKV
projection for its 8 q-heads + its kv head (k and v columns), applies RoPE,
and runs causal GQA attention for its 8 heads over the full sequence.
No collectives; per-core inputs/outputs are sharded and assembled on host.

Device layout is "transposed": qkvT [n, s] with head-dim on partitions, so
the projection needs no transposes (lhsT = w columns, rhs = hiddenT) and
scores come out as scoresT [k, s_q] whose softmax sum is done with an
ones-vector matmul. All matmuls run in float32r (TF32-like, full fp32
storage, ~2e-4 matmul error).
"""
import numpy as np
from contextlib import ExitStack

import concourse.bacc as bacc
import concourse.tile as tile
import concourse.mybir as mybir

# Problem constants (hardcoded per contract)
B, S, HIDDEN = 2, 2048, 4096
NUM_HEADS, NUM_KV_HEADS, D = 32, 2, 128
ROPE_BASE = 10000.0
N_CORES = 8
HEADS_PER_CORE = NUM_HEADS // 4          # 8 (TP=4)
NC_CHUNKS = HEADS_PER_CORE + 2           # 8 q + 1 k + 1 v = 10 n-chunks of 128
SC = 512                                 # s-chunk (psum bank = 512 fp32)
NSC = S // SC                            # 4
NKT = S // 128                           # 16 k-tiles
SCALE = float(D) ** -0.5
BIG = 30000.0

f32 = mybir.dt.float32
f32r = mybir.dt.float32r

_CACHE: dict = {}


def _build_nc():
    nc = bacc.Bacc(trn_type="TRN2", target_bir_lowering=False, debug=False)

    hT_d = nc.dram_tensor("hT", [HIDDEN, S], f32, kind="ExternalInput").ap()
    # wc pre-tiled on host: wc_t[n*128+p, k*128+m] = w_slice[k*128+p, n*128+m]
    wc_d = nc.dram_tensor("wc", [NC_CHUNKS * 128, HIDDEN], f32, kind="ExternalInput").ap()
    cosF_d = nc.dram_tensor("cosF", [128, S], f32, kind="ExternalInput").ap()
    sinS_d = nc.dram_tensor("sinS", [128, S], f32, kind="ExternalInput").ap()
    ident_d = nc.dram_tensor("ident", [128, 128], f32, kind="ExternalInput").ap()
    perm_d = nc.dram_tensor("perm", [128, 128], f32, kind="ExternalInput").ap()
    trione_d = nc.dram_tensor("trione", [128, 128], f32, kind="ExternalInput").ap()
    maskB_d = nc.dram_tensor("maskB", [128, 4 * SC], f32, kind="ExternalInput").ap()
    onesc_d = nc.dram_tensor("onesc", [128, 1], f32, kind="ExternalInput").ap()
    onesr_d = nc.dram_tensor("onesr", [1, 128], f32, kind="ExternalInput").ap()
    outT_d = nc.dram_tensor("outT", [HEADS_PER_CORE * 128, S], f32, kind="ExternalOutput").ap()

    with tile.TileContext(nc) as tc, ExitStack() as ctx:
        cpool = ctx.enter_context(tc.tile_pool(name="consts", bufs=1))
        qk_pool = ctx.enter_context(tc.tile_pool(name="qkt", bufs=1))
        v_pool = ctx.enter_context(tc.tile_pool(name="vsb", bufs=1))

        ident = cpool.tile([128, 128], f32r, tag="ident")
        perm = cpool.tile([128, 128], f32r, tag="perm")
        trione = cpool.tile([128, 128], f32r, tag="trione")
        maskB = cpool.tile([128, 4 * SC], f32r, tag="maskB")
        ones_c = cpool.tile([128, 1], f32r, tag="onesc")
        ones_r = cpool.tile([1, 128], f32r, tag="onesr")
        nc.sync.dma_start(ident[:], ident_d.bitcast(f32r))
        nc.sync.dma_start(perm[:], perm_d.bitcast(f32r))
        nc.sync.dma_start(trione[:], trione_d.bitcast(f32r))
        nc.sync.dma_start(maskB[:], maskB_d.bitcast(f32r))
        nc.sync.dma_start(ones_c[:], onesc_d.bitcast(f32r))
        nc.sync.dma_start(ones_r[:], onesr_d.bitcast(f32r))

        # persistent: 8 q heads + k, all RoPE'd, [d, s] layout
        qkT = [
            qk_pool.tile([128, S], f32r, tag=f"qkT{n}", name=f"qkT{n}")
            for n in range(9)
        ]
        # v in [s, d] layout: tile t at columns t*128:(t+1)*128
        v_sb = v_pool.tile([128, NKT * 128], f32r, tag="vsb")

        # ---------------- projection + RoPE ----------------
        with ExitStack() as pctx:
            hts_pool = pctx.enter_context(tc.tile_pool(name="hts", bufs=32))
            w_pool = pctx.enter_context(tc.tile_pool(name="wt", bufs=2))
            tab_pool = pctx.enter_context(tc.tile_pool(name="tabs", bufs=2))
            rope_pool = pctx.enter_context(tc.tile_pool(name="rope", bufs=2))
            pp = pctx.enter_context(tc.tile_pool(name="pp", bufs=3, space="PSUM"))
            swp = pctx.enter_context(tc.tile_pool(name="swp", bufs=2, space="PSUM"))
            vtp = pctx.enter_context(tc.tile_pool(name="vtp", bufs=2, space="PSUM"))

            for sc in range(NSC):
                ssl = slice(sc * SC, (sc + 1) * SC)
                ht = []
                for k in range(HIDDEN // 128):
                    t = hts_pool.tile([128, SC], f32r, tag="ht")
                    nc.sync.dma_start(t[:], hT_d[k * 128:(k + 1) * 128, ssl].bitcast(f32r))
                    ht.append(t)
                cos_t = tab_pool.tile([128, SC], f32, tag="cos")
                sin_t = tab_pool.tile([128, SC], f32, tag="sin")
                nc.sync.dma_start(cos_t[:], cosF_d[:, ssl])
                nc.sync.dma_start(sin_t[:], sinS_d[:, ssl])

                for n in range(NC_CHUNKS):
                    wn = w_pool.tile([128, HIDDEN], f32r, tag="wn")
                    # wn[:, k*128:(k+1)*128] = w_slice[k*128:(k+1)*128, n-cols]
                    nc.sync.dma_start(
                        wn[:], wc_d[n * 128:(n + 1) * 128, :].bitcast(f32r)
                    )
                    psum = pp.tile([128, SC], f32, tag="proj")
                    for k in range(HIDDEN // 128):
                        nc.tensor.matmul(
                            psum[:], wn[:, k * 128:(k + 1) * 128], ht[k][:],
                            start=(k == 0), stop=(k == HIDDEN // 128 - 1),
                        )
                    if n < 9:
                        # RoPE: out = raw*cos + swap(raw)*sin_signed
                        qraw = rope_pool.tile([128, SC], f32r, tag="qraw")
                        nc.scalar.copy(qraw[:], psum[:])
                        swps = swp.tile([128, SC], f32, tag="swp")
                        nc.tensor.matmul(swps[:], perm[:], qraw[:], start=True, stop=True)
                        t1 = rope_pool.tile([128, SC], f32, tag="t1")
                        nc.vector.tensor_mul(t1[:], qraw[:].bitcast(f32), cos_t[:])
                        t2 = rope_pool.tile([128, SC], f32, tag="t2")
                        nc.vector.tensor_mul(t2[:], swps[:], sin_t[:])
                        with nc.allow_low_precision(reason="fp32r rounding of rope output"):
                            nc.vector.tensor_add(qkT[n][:, ssl], t1[:], t2[:])
                    else:
                        vraw = rope_pool.tile([128, SC], f32r, tag="qraw")
                        nc.scalar.copy(vraw[:], psum[:])
                        for j in range(SC // 128):
                            vt = vtp.tile([128, 128], f32r, tag="vt")
                            with nc.allow_low_precision(reason="fp32r transpose"):
                                nc.tensor.transpose(
                                    vt[:], vraw[:, j * 128:(j + 1) * 128], ident[:]
                                )
                            kt_glob = sc * (SC // 128) + j
                            nc.vector.tensor_copy(
                                v_sb[:, kt_glob * 128:(kt_glob + 1) * 128], vt[:]
                            )

        # ---------------- attention ----------------
        with ExitStack() as actx:
            probs_pool = actx.enter_context(tc.tile_pool(name="probs", bufs=4))
            att_pool = actx.enter_context(tc.tile_pool(name="att", bufs=2))
            scp = actx.enter_context(tc.tile_pool(name="scp", bufs=2, space="PSUM"))
            pvp = actx.enter_context(tc.tile_pool(name="pvp", bufs=2, space="PSUM"))
            lxp = actx.enter_context(tc.tile_pool(name="lxp", bufs=2, space="PSUM"))

            kT = qkT[8]
            for h in range(HEADS_PER_CORE):
                for qc in range(NSC):
                    n_kt = (qc + 1) * (SC // 128)
                    pv = pvp.tile([128, SC], f32, tag="pv")
                    lacc = pvp.tile([1, SC], f32, tag="lacc")
                    for kt in range(n_kt):
                        oo = kt * 128 - qc * SC
                        is_partial = 0 <= oo < SC
                        scps = scp.tile([128, SC], f32, tag="sc")
                        nc.tensor.matmul(
                            scps[:], kT[:, kt * 128:(kt + 1) * 128],
                            qkT[h][:, qc * SC:(qc + 1) * SC],
                            start=True, stop=not is_partial,
                        )
                        if is_partial:
                            nc.tensor.matmul(
                                scps[:], trione[:],
                                maskB[:, (oo // 128) * SC:(oo // 128 + 1) * SC],
                                start=False, stop=True,
                            )
                        probs = probs_pool.tile([128, SC], f32r, tag="probs")
                        nc.scalar.activation(
                            probs[:], scps[:], mybir.ActivationFunctionType.Exp,
                            scale=SCALE,
                        )
                        nc.tensor.matmul(
                            pv[:], v_sb[:, kt * 128:(kt + 1) * 128], probs[:],
                            start=(kt == 0), stop=(kt == n_kt - 1),
                        )
                        nc.tensor.matmul(
                            lacc[:], ones_c[:], probs[:],
                            start=(kt == 0), stop=(kt == n_kt - 1),
                        )
                    lrec = att_pool.tile([1, SC], f32r, tag="lrec")
                    with nc.allow_low_precision(reason="softmax denom recip"):
                        nc.vector.reciprocal(lrec[:], lacc[:])
                    lexp_ps = lxp.tile([128, SC], f32, tag="lexp")
                    nc.tensor.matmul(lexp_ps[:], ones_r[:], lrec[:], start=True, stop=True)
                    lexp = att_pool.tile([128, SC], f32, tag="lexpsb")
                    nc.scalar.copy(lexp[:], lexp_ps[:])
                    outn = att_pool.tile([128, SC], f32, tag="outn")
                    nc.vector.tensor_mul(outn[:], pv[:], lexp[:])
                    nc.sync.dma_start(
                        outT_d[h * 128:(h + 1) * 128, qc * SC:(qc + 1) * SC], outn[:]
                    )

    nc.finalize()
    return nc


def _get_runner():
    """Build nc once and a cached jitted shard_map callable (axon/PJRT)."""
    if "runner" in _CACHE:
        return _CACHE["runner"]

    import jax
    import jax.numpy as jnp  # noqa: F401
    from jax.sharding import Mesh, PartitionSpec
    from jax.experimental.shard_map import shard_map
    from concourse.bass2jax import (
        install_neuronx_cc_hook, _bass_exec_p, partition_id_tensor,
    )
    import concourse.mybir as _mybir

    nc = _build_nc()
    install_neuronx_cc_hook()

    partition_name = nc.partition_id_tensor.name if nc.partition_id_tensor else None
    in_names, out_names, out_avals, zero_outs = [], [], [], []
    for alloc in nc.m.functions[0].allocations:
        if not isinstance(alloc, _mybir.MemoryLocationSet):
            continue
        name = alloc.memorylocations[0].name
        if alloc.kind == "ExternalInput":
            if name != partition_name:
                in_names.append(name)
        elif alloc.kind == "ExternalOutput":
            shape = tuple(alloc.tensor_shape)
            npdt = _mybir.dt.np(alloc.dtype)
            out_avals.append(jax.core.ShapedArray(shape, npdt))
            out_names.append(name)
            zero_outs.append(np.zeros(shape, npdt))

    n_params = len(in_names)
    n_outs = len(out_avals)
    all_in_names = in_names + out_names
    if partition_name is not None:
        all_in_names.append(partition_name)
    donate = tuple(range(n_params, n_params + n_outs))

    def _body(*args):
        operands = list(args)
        if partition_name is not None:
            operands.append(partition_id_tensor())
        outs = _bass_exec_p.bind(
            *operands,
            out_avals=tuple(out_avals),
            in_names=tuple(all_in_names),
            out_names=tuple(out_names),
            lowering_input_output_aliases=(),
            sim_require_finite=True,
            sim_require_nnan=True,
            nc=nc,
        )
        return tuple(outs)

    devices = jax.devices()[:N_CORES]
    mesh = Mesh(np.asarray(devices), ("core",))
    in_specs = (PartitionSpec("core"),) * (n_params + n_outs)
    out_specs = (PartitionSpec("core"),) * n_outs
    fn = jax.jit(
        shard_map(_body, mesh=mesh, in_specs=in_specs, out_specs=out_specs,
                  check_rep=False),
        donate_argnums=donate,
        keep_unused=True,
    )

    runner = (fn, in_names, out_names, out_avals, zero_outs)
    _CACHE["runner"] = runner
    return runner


def _host_prep(positions, hidden_states, w_qkv):
    """Build the per-core input maps (shard + layout prep, no reference math)."""
    positions = np.asarray(positions)
    hidden_states = np.ascontiguousarray(np.asarray(hidden_states, dtype=np.float32))
    w_qkv = np.ascontiguousarray(np.asarray(w_qkv, dtype=np.float32))

    half = D // 2
    inv_freq = 1.0 / (ROPE_BASE ** (np.arange(half, dtype=np.float32) / half))
    ang = positions.astype(np.float32)[:, None] * inv_freq[None, :]  # [S, 64]
    cos = np.cos(ang).astype(np.float32)  # [S, 64]
    sin = np.sin(ang).astype(np.float32)
    cosF = np.empty((128, S), np.float32)
    sinS = np.empty((128, S), np.float32)
    cosF[:half] = cos.T
    cosF[half:] = cos.T
    sinS[:half] = -sin.T
    sinS[half:] = sin.T

    ident = np.eye(128, dtype=np.float32)
    perm = np.roll(np.eye(128, dtype=np.float32), 64, axis=0)
    trione = np.triu(np.ones((128, 128), np.float32))
    maskB = np.zeros((4, 128, SC), np.float32)
    for oi, o in enumerate([0, 128, 256, 384]):
        for c in range(1, 128):
            qq = c + o - 1
            if 0 <= qq < SC:
                maskB[oi, c, qq] = -BIG
        maskB[oi, 0, :o] = -BIG
    maskB_flat = np.ascontiguousarray(
        maskB.transpose(1, 0, 2).reshape(128, 4 * SC)
    )
    onesc = np.ones((128, 1), np.float32)
    onesr = np.ones((1, 128), np.float32)

    hT = [np.ascontiguousarray(hidden_states[b].T) for b in range(B)]

    q_sz = NUM_HEADS * D
    in_maps = []
    for c in range(N_CORES):
        b, tp = divmod(c, 4)
        kv = tp // 2
        wq = w_qkv[:, tp * 1024:(tp + 1) * 1024]
        wk = w_qkv[:, q_sz + kv * 128: q_sz + (kv + 1) * 128]
        wv = w_qkv[:, q_sz + NUM_KV_HEADS * D + kv * 128:
                      q_sz + NUM_KV_HEADS * D + (kv + 1) * 128]
        wc = np.concatenate([wq, wk, wv], axis=1)  # [4096, 1280]
        # tile to [n*128+p, k*128+m] = wc[k*128+p, n*128+m]
        wc_t = np.ascontiguousarray(
            wc.reshape(HIDDEN // 128, 128, NC_CHUNKS, 128)
            .transpose(2, 1, 0, 3)
            .reshape(NC_CHUNKS * 128, HIDDEN)
        )
        in_maps.append({
            "hT": hT[b], "wc": wc_t, "cosF": cosF, "sinS": sinS,
            "ident": ident, "perm": perm, "trione": trione,
            "maskB": maskB_flat, "onesc": onesc, "onesr": onesr,
        })
    return in_maps


def run_device(in_maps):
    """Run the compiled kernel on 8 cores; returns list of per-core outputs."""
    fn, in_names, out_names, out_avals, zero_outs = _get_runner()
    per_core = [[np.asarray(m[nm]) for nm in in_names] for m in in_maps]
    concat_in = [
        np.concatenate([per_core[c][i] for c in range(N_CORES)], axis=0)
        for i in range(len(in_names))
    ]
    concat_zeros = [
        np.zeros((N_CORES * z.shape[0], *z.shape[1:]), z.dtype) for z in zero_outs
    ]
    out_arrs = fn(*concat_in, *concat_zeros)
    return [
        {
            nm: np.asarray(out_arrs[i]).reshape(N_CORES, *out_avals[i].shape)[c]
            for i, nm in enumerate(out_names)
        }
        for c in range(N_CORES)
    ]


def kernel(positions, hidden_states, w_qkv):
    in_maps = _host_prep(positions, hidden_states, w_qkv)
    results = run_device(in_maps)
    out = np.empty((B, S, NUM_HEADS * D), np.float32)
    for c in range(N_CORES):
        b, tp = divmod(c, 4)
        oT = results[c]["outT"].reshape(HEADS_PER_CORE, 128, S)
        out[b, :, tp * 1024:(tp + 1) * 1024] = (
            oT.transpose(2, 0, 1).reshape(S, HEADS_PER_CORE * 128)
        )
    return out


# revision 10
# speedup vs baseline: 84.7804x; 84.7804x over previous
"""ChatGLM3 attention (B=2, S=2048, H=4096, 32 q-heads / 2 kv-heads, D=128)
on 8 Trainium2 NeuronCores.

Sharding: core c = 4*b + tp  (b in {0,1} data-parallel over batch,
tp in {0..3} tensor-parallel over heads). Each core computes the QKV
projection for its 8 q-heads + its kv head (k and v columns), applies RoPE,
and runs causal GQA attention for its 8 heads over the full sequence.
No collectives; per-core inputs/outputs are sharded and assembled on host.

Device layout is "transposed": qkvT [n, s] with head-dim on partitions, so
the projection needs no transposes (lhsT = w columns, rhs = hiddenT) and
scores come out as scoresT [k, s_q] whose softmax sum is done with an
ones-vector matmul. All matmuls run in float32r (TF32-like, full fp32
storage, ~2e-4 matmul error).
"""
import numpy as np
from contextlib import ExitStack

import concourse.bacc as bacc
import concourse.tile as tile
import concourse.mybir as mybir

# Problem constants (hardcoded per contract)
B, S, HIDDEN = 2, 2048, 4096
NUM_HEADS, NUM_KV_HEADS, D = 32, 2, 128
ROPE_BASE = 10000.0
N_CORES = 8
HEADS_PER_CORE = NUM_HEADS // 4          # 8 (TP=4)
NC_CHUNKS = HEADS_PER_CORE + 2           # 8 q + 1 k + 1 v = 10 n-chunks of 128
SC = 512                                 # s-chunk (psum bank = 512 fp32)
NSC = S // SC                            # 4
NKT = S // 128                           # 16 k-tiles
SCALE = float(D) ** -0.5
BIG = 30000.0

f32 = mybir.dt.float32
f32r = mybir.dt.float32r

_CACHE: dict = {}


def _build_nc(loop_n: int = 1, parts=("proj", "attn")):
    nc = bacc.Bacc(trn_type="TRN2", target_bir_lowering=False, debug=False)

    hT_d = nc.dram_tensor("hT", [HIDDEN, S], f32, kind="ExternalInput").ap()
    # wc pre-tiled on host: wc_t[n*128+p, k*128+m] = w_slice[k*128+p, n*128+m]
    wc_d = nc.dram_tensor("wc", [NC_CHUNKS * 128, HIDDEN], f32, kind="ExternalInput").ap()
    cosF_d = nc.dram_tensor("cosF", [128, S], f32, kind="ExternalInput").ap()
    sinS_d = nc.dram_tensor("sinS", [128, S], f32, kind="ExternalInput").ap()
    ident_d = nc.dram_tensor("ident", [128, 128], f32, kind="ExternalInput").ap()
    perm_d = nc.dram_tensor("perm", [128, 128], f32, kind="ExternalInput").ap()
    trione_d = nc.dram_tensor("trione", [128, 128], f32, kind="ExternalInput").ap()
    maskB_d = nc.dram_tensor("maskB", [128, 4 * SC], f32, kind="ExternalInput").ap()
    onesc_d = nc.dram_tensor("onesc", [128, 1], f32, kind="ExternalInput").ap()
    onesr_d = nc.dram_tensor("onesr", [1, 128], f32, kind="ExternalInput").ap()
    outT_d = nc.dram_tensor("outT", [HEADS_PER_CORE * 128, S], f32, kind="ExternalOutput").ap()

    with tile.TileContext(nc) as tc, ExitStack() as ctx:
        if loop_n > 1:
            ctx.enter_context(tc.For_i(0, loop_n, 1))
        cpool = ctx.enter_context(tc.tile_pool(name="consts", bufs=1))
        qk_pool = ctx.enter_context(tc.tile_pool(name="qkt", bufs=1))
        v_pool = ctx.enter_context(tc.tile_pool(name="vsb", bufs=1))

        ident = cpool.tile([128, 128], f32r, tag="ident")
        perm = cpool.tile([128, 128], f32r, tag="perm")
        trione = cpool.tile([128, 128], f32r, tag="trione")
        maskB = cpool.tile([128, 4 * SC], f32r, tag="maskB")
        ones_c = cpool.tile([128, 1], f32r, tag="onesc")
        ones_r = cpool.tile([1, 128], f32r, tag="onesr")
        nc.sync.dma_start(ident[:], ident_d.bitcast(f32r))
        nc.sync.dma_start(perm[:], perm_d.bitcast(f32r))
        nc.sync.dma_start(trione[:], trione_d.bitcast(f32r))
        nc.sync.dma_start(maskB[:], maskB_d.bitcast(f32r))
        nc.sync.dma_start(ones_c[:], onesc_d.bitcast(f32r))
        nc.sync.dma_start(ones_r[:], onesr_d.bitcast(f32r))

        # persistent: 8 q heads + k, all RoPE'd, [d, s] layout
        qkT = [
            qk_pool.tile([128, S], f32r, tag=f"qkT{n}", name=f"qkT{n}")
            for n in range(9)
        ]
        # v in [s, d] layout: tile t at columns t*128:(t+1)*128
        v_sb = v_pool.tile([128, NKT * 128], f32r, tag="vsb")

        # ---------------- projection + RoPE ----------------
        if "proj" not in parts:
            nc.sync.dma_start(qkT[0][:, 0:SC], hT_d[0:128, 0:SC].bitcast(f32r))
        with ExitStack() as pctx:
          if "proj" in parts:
            hts_pool = pctx.enter_context(tc.tile_pool(name="hts", bufs=32))
            w_pool = pctx.enter_context(tc.tile_pool(name="wt", bufs=2))
            tab_pool = pctx.enter_context(tc.tile_pool(name="tabs", bufs=2))
            rope_pool = pctx.enter_context(tc.tile_pool(name="rope", bufs=2))
            pp = pctx.enter_context(tc.tile_pool(name="pp", bufs=3, space="PSUM"))
            swp = pctx.enter_context(tc.tile_pool(name="swp", bufs=2, space="PSUM"))
            vtp = pctx.enter_context(tc.tile_pool(name="vtp", bufs=2, space="PSUM"))

            for sc in (range(NSC) if "proj" in parts else []):
                ssl = slice(sc * SC, (sc + 1) * SC)
                ht = []
                for k in range(HIDDEN // 128):
                    t = hts_pool.tile([128, SC], f32r, tag="ht")
                    if "nohdma" in parts:
                        nc.gpsimd.memset(t[:], 0.0)
                    else:
                        nc.sync.dma_start(t[:], hT_d[k * 128:(k + 1) * 128, ssl].bitcast(f32r))
                    ht.append(t)
                cos_t = tab_pool.tile([128, SC], f32, tag="cos")
                sin_t = tab_pool.tile([128, SC], f32, tag="sin")
                nc.sync.dma_start(cos_t[:], cosF_d[:, ssl])
                nc.sync.dma_start(sin_t[:], sinS_d[:, ssl])

                for n in range(NC_CHUNKS):
                    wn = w_pool.tile([128, HIDDEN], f32r, tag="wn")
                    # wn[:, k*128:(k+1)*128] = w_slice[k*128:(k+1)*128, n-cols]
                    if "nowdma" in parts:
                        nc.gpsimd.memset(wn[:], 0.0)
                    else:
                        nc.sync.dma_start(
                            wn[:], wc_d[n * 128:(n + 1) * 128, :].bitcast(f32r)
                        )
                    psum = pp.tile([128, SC], f32, tag="proj")
                    for k in range(HIDDEN // 128):
                        nc.tensor.matmul(
                            psum[:], wn[:, k * 128:(k + 1) * 128], ht[k][:],
                            start=(k == 0), stop=(k == HIDDEN // 128 - 1),
                        )
                    if n < 9:
                        # RoPE: out = raw*cos + swap(raw)*sin_signed
                        qraw = rope_pool.tile([128, SC], f32r, tag="qraw")
                        nc.scalar.copy(qraw[:], psum[:])
                        swps = swp.tile([128, SC], f32, tag="swp")
                        nc.tensor.matmul(swps[:], perm[:], qraw[:], start=True, stop=True)
                        t1 = rope_pool.tile([128, SC], f32, tag="t1")
                        nc.vector.tensor_mul(t1[:], qraw[:].bitcast(f32), cos_t[:])
                        t2 = rope_pool.tile([128, SC], f32, tag="t2")
                        nc.vector.tensor_mul(t2[:], swps[:], sin_t[:])
                        with nc.allow_low_precision(reason="fp32r rounding of rope output"):
                            nc.vector.tensor_add(qkT[n][:, ssl], t1[:], t2[:])
                    else:
                        vraw = rope_pool.tile([128, SC], f32r, tag="qraw")
                        nc.scalar.copy(vraw[:], psum[:])
                        for j in range(SC // 128):
                            vt = vtp.tile([128, 128], f32r, tag="vt")
                            with nc.allow_low_precision(reason="fp32r transpose"):
                                nc.tensor.transpose(
                                    vt[:], vraw[:, j * 128:(j + 1) * 128], ident[:]
                                )
                            kt_glob = sc * (SC // 128) + j
                            nc.vector.tensor_copy(
                                v_sb[:, kt_glob * 128:(kt_glob + 1) * 128], vt[:]
                            )

        # ---------------- attention ----------------
        with ExitStack() as actx:
          if True:
            probs_pool = actx.enter_context(tc.tile_pool(name="probs", bufs=4))
            att_pool = actx.enter_context(tc.tile_pool(name="att", bufs=2))
            scp = actx.enter_context(tc.tile_pool(name="scp", bufs=2, space="PSUM"))
            pvp = actx.enter_context(tc.tile_pool(name="pvp", bufs=2, space="PSUM"))
            lxp = actx.enter_context(tc.tile_pool(name="lxp", bufs=2, space="PSUM"))

            kT = qkT[8]
            for h in (range(HEADS_PER_CORE) if "attn" in parts else []):
                for qc in range(NSC):
                    n_kt = (qc + 1) * (SC // 128)
                    pv = pvp.tile([128, SC], f32, tag="pv")
                    lacc = pvp.tile([1, SC], f32, tag="lacc")
                    for kt in range(n_kt):
                        oo = kt * 128 - qc * SC
                        is_partial = 0 <= oo < SC
                        scps = scp.tile([128, SC], f32, tag="sc")
                        nc.tensor.matmul(
                            scps[:], kT[:, kt * 128:(kt + 1) * 128],
                            qkT[h][:, qc * SC:(qc + 1) * SC],
                            start=True, stop=not is_partial,
                        )
                        if is_partial:
                            nc.tensor.matmul(
                                scps[:], trione[:],
                                maskB[:, (oo // 128) * SC:(oo // 128 + 1) * SC],
                                start=False, stop=True,
                            )
                        probs = probs_pool.tile([128, SC], f32r, tag="probs")
                        nc.scalar.activation(
                            probs[:], scps[:], mybir.ActivationFunctionType.Exp,
                            scale=SCALE,
                        )
                        nc.tensor.matmul(
                            pv[:], v_sb[:, kt * 128:(kt + 1) * 128], probs[:],
                            start=(kt == 0), stop=(kt == n_kt - 1),
                        )
                        nc.tensor.matmul(
                            lacc[:], ones_c[:], probs[:],
                            start=(kt == 0), stop=(kt == n_kt - 1),
                        )
                    lrec = att_pool.tile([1, SC], f32r, tag="lrec")
                    with nc.allow_low_precision(reason="softmax denom recip"):
                        nc.vector.reciprocal(lrec[:], lacc[:])
                    lexp_ps = lxp.tile([128, SC], f32, tag="lexp")
                    nc.tensor.matmul(lexp_ps[:], ones_r[:], lrec[:], start=True, stop=True)
                    lexp = att_pool.tile([128, SC], f32, tag="lexpsb")
                    nc.scalar.copy(lexp[:], lexp_ps[:])
                    outn = att_pool.tile([128, SC], f32, tag="outn")
                    nc.vector.tensor_mul(outn[:], pv[:], lexp[:])
                    nc.sync.dma_start(
                        outT_d[h * 128:(h + 1) * 128, qc * SC:(qc + 1) * SC], outn[:]
                    )

    nc.finalize()
    return nc


def _get_runner(loop_n: int = 1):
    """Build nc once and a cached jitted shard_map callable (axon/PJRT)."""
    key = f"runner{loop_n}"
    if key in _CACHE:
        return _CACHE[key]

    import jax
    import jax.numpy as jnp  # noqa: F401
    from jax.sharding import Mesh, PartitionSpec
    from jax.experimental.shard_map import shard_map
    from concourse.bass2jax import (
        install_neuronx_cc_hook, _bass_exec_p, partition_id_tensor,
    )
    import concourse.mybir as _mybir

    nc = _build_nc(loop_n)
    install_neuronx_cc_hook()

    partition_name = nc.partition_id_tensor.name if nc.partition_id_tensor else None
    in_names, out_names, out_avals, zero_outs = [], [], [], []
    for alloc in nc.m.functions[0].allocations:
        if not isinstance(alloc, _mybir.MemoryLocationSet):
            continue
        name = alloc.memorylocations[0].name
        if alloc.kind == "ExternalInput":
            if name != partition_name:
                in_names.append(name)
        elif alloc.kind == "ExternalOutput":
            shape = tuple(alloc.tensor_shape)
            npdt = _mybir.dt.np(alloc.dtype)
            out_avals.append(jax.core.ShapedArray(shape, npdt))
            out_names.append(name)
            zero_outs.append(np.zeros(shape, npdt))

    n_params = len(in_names)
    n_outs = len(out_avals)
    all_in_names = in_names + out_names
    if partition_name is not None:
        all_in_names.append(partition_name)
    donate = tuple(range(n_params, n_params + n_outs))

    def _body(*args):
        operands = list(args)
        if partition_name is not None:
            operands.append(partition_id_tensor())
        outs = _bass_exec_p.bind(
            *operands,
            out_avals=tuple(out_avals),
            in_names=tuple(all_in_names),
            out_names=tuple(out_names),
            lowering_input_output_aliases=(),
            sim_require_finite=True,
            sim_require_nnan=True,
            nc=nc,
        )
        return tuple(outs)

    devices = jax.devices()[:N_CORES]
    mesh = Mesh(np.asarray(devices), ("core",))
    in_specs = (PartitionSpec("core"),) * (n_params + n_outs)
    out_specs = (PartitionSpec("core"),) * n_outs
    fn = jax.jit(
        shard_map(_body, mesh=mesh, in_specs=in_specs, out_specs=out_specs,
                  check_rep=False),
        donate_argnums=donate,
        keep_unused=True,
    )

    runner = (fn, in_names, out_names, out_avals, zero_outs)
    _CACHE[key] = runner
    return runner


def _host_prep(positions, hidden_states, w_qkv):
    """Build the per-core input maps (shard + layout prep, no reference math)."""
    positions = np.asarray(positions)
    hidden_states = np.ascontiguousarray(np.asarray(hidden_states, dtype=np.float32))
    w_qkv = np.ascontiguousarray(np.asarray(w_qkv, dtype=np.float32))

    half = D // 2
    inv_freq = 1.0 / (ROPE_BASE ** (np.arange(half, dtype=np.float32) / half))
    ang = positions.astype(np.float32)[:, None] * inv_freq[None, :]  # [S, 64]
    cos = np.cos(ang).astype(np.float32)  # [S, 64]
    sin = np.sin(ang).astype(np.float32)
    cosF = np.empty((128, S), np.float32)
    sinS = np.empty((128, S), np.float32)
    cosF[:half] = cos.T
    cosF[half:] = cos.T
    sinS[:half] = -sin.T
    sinS[half:] = sin.T

    ident = np.eye(128, dtype=np.float32)
    perm = np.roll(np.eye(128, dtype=np.float32), 64, axis=0)
    trione = np.triu(np.ones((128, 128), np.float32))
    maskB = np.zeros((4, 128, SC), np.float32)
    for oi, o in enumerate([0, 128, 256, 384]):
        for c in range(1, 128):
            qq = c + o - 1
            if 0 <= qq < SC:
                maskB[oi, c, qq] = -BIG
        maskB[oi, 0, :o] = -BIG
    maskB_flat = np.ascontiguousarray(
        maskB.transpose(1, 0, 2).reshape(128, 4 * SC)
    )
    onesc = np.ones((128, 1), np.float32)
    onesr = np.ones((1, 128), np.float32)

    hT = [np.ascontiguousarray(hidden_states[b].T) for b in range(B)]

    q_sz = NUM_HEADS * D
    in_maps = []
    for c in range(N_CORES):
        b, tp = divmod(c, 4)
        kv = tp // 2
        wq = w_qkv[:, tp * 1024:(tp + 1) * 1024]
        wk = w_qkv[:, q_sz + kv * 128: q_sz + (kv + 1) * 128]
        wv = w_qkv[:, q_sz + NUM_KV_HEADS * D + kv * 128:
                      q_sz + NUM_KV_HEADS * D + (kv + 1) * 128]
        wc = np.concatenate([wq, wk, wv], axis=1)  # [4096, 1280]
        # tile to [n*128+p, k*128+m] = wc[k*128+p, n*128+m]
        wc_t = np.ascontiguousarray(
            wc.reshape(HIDDEN // 128, 128, NC_CHUNKS, 128)
            .transpose(2, 1, 0, 3)
            .reshape(NC_CHUNKS * 128, HIDDEN)
        )
        in_maps.append({
            "hT": hT[b], "wc": wc_t, "cosF": cosF, "sinS": sinS,
            "ident": ident, "perm": perm, "trione": trione,
            "maskB": maskB_flat, "onesc": onesc, "onesr": onesr,
        })
    return in_maps


def run_device(in_maps):
    """Run the compiled kernel on 8 cores; returns list of per-core outputs."""
    fn, in_names, out_names, out_avals, zero_outs = _get_runner()
    per_core = [[np.asarray(m[nm]) for nm in in_names] for m in in_maps]
    concat_in = [
        np.concatenate([per_core[c][i] for c in range(N_CORES)], axis=0)
        for i in range(len(in_names))
    ]
    concat_zeros = [
        np.zeros((N_CORES * z.shape[0], *z.shape[1:]), z.dtype) for z in zero_outs
    ]
    out_arrs = fn(*concat_in, *concat_zeros)
    return [
        {
            nm: np.asarray(out_arrs[i]).reshape(N_CORES, *out_avals[i].shape)[c]
            for i, nm in enumerate(out_names)
        }
        for c in range(N_CORES)
    ]


def kernel(positions, hidden_states, w_qkv):
    in_maps = _host_prep(positions, hidden_states, w_qkv)
    results = run_device(in_maps)
    out = np.empty((B, S, NUM_HEADS * D), np.float32)
    for c in range(N_CORES):
        b, tp = divmod(c, 4)
        oT = results[c]["outT"].reshape(HEADS_PER_CORE, 128, S)
        out[b, :, tp * 1024:(tp + 1) * 1024] = (
            oT.transpose(2, 0, 1).reshape(S, HEADS_PER_CORE * 128)
        )
    return out
